# revision 13
# baseline (speedup 1.0000x reference)
"""Trainium2 Bass kernel for nn_DepthwiseXCorr (SiamRPN++-style depthwise-xcorr head).

Pipeline per sample (data-parallel over batch: 64 samples -> 8 cores x 8):
  conv3x3(kernel,wk)+BN+ReLU -> k_feat [256,5,5]
  conv3x3(search,ws)+BN+ReLU -> s_feat [256,29,29]   (bf16, 29-wide rows)
  depthwise xcorr(s_feat,k_feat) -> feat [256,25,25]
  1x1 conv w1 + BN + ReLU -> h [256,25,25]
  1x1 conv w2 + b2 -> out [20,25,25]

Convs run in bf16 (no fp32r even-size restriction -> no garbage columns);
the head 1x1 convs run in fp32r. The depthwise xcorr is split across
engines per (sample, og) chain of 25 taps:
  - PE: per-tap diagonal-weight matmuls (bf16 diags built on ACT, two
    samples ahead so PE never waits)
  - Pool (GPSIMD): tensor_tensor adds of pre-scaled windows (products from
    ACT scaled-copies or DVE fast tensor_scalar)
  - DVE: scalar_tensor_tensor multiply-accumulate taps
Partial sums chain through one f32 accumulator (ACT copies the PE PSUM
partial in, Pool then DVE accumulate on top); the final tap writes the
f32r head input. Emission is software-pipelined: front(s), stt(s-1),
head(s-3); the last sample is all-PE so the drain is short.
"""
import numpy as np

EPS = 1e-5
NCORES = 8
BPC = 8          # samples per core

# per og-sample (idx = 2*s + og): (p, qa, qd, d)
#   p taps on PE, qa products on ACT -> Pool adds, qd products on DVE ->
#   Pool adds, d taps on DVE STT. p+qa+qd+d = 25.
SCHED = [
    (0, 9, 4, 12), (0, 9, 4, 12),
    (5, 4, 2, 14), (5, 4, 2, 14),
    (5, 4, 2, 14), (5, 4, 2, 14),
    (5, 4, 2, 14), (5, 4, 2, 14),
    (6, 3, 2, 14), (6, 3, 2, 14),
    (7, 4, 2, 12), (7, 4, 2, 12),
    (8, 4, 2, 11), (9, 4, 2, 10),
    (16, 5, 1, 3), (25, 0, 0, 0),
]

_CACHE = {}


def _shift_window(ap_2d, base_off, rows, cols, rowstride):
    """AP reading [128, rows, cols] window at element offset base_off of a
    [128, W] SBUF view, row stride in elements."""
    import concourse.bass as bass
    return bass.AP(
        tensor=ap_2d.tensor,
        offset=ap_2d.offset + base_off,
        ap=[list(ap_2d.ap[0]), [rowstride, rows], [1, cols]],
    )


def _build(bench_R=0):
    import concourse.bacc as bacc
    import concourse.bass as bass
    import concourse.mybir as mybir
    import concourse.tile as tile

    f32 = mybir.dt.float32
    f32r = mybir.dt.float32r
    bf16 = mybir.dt.bfloat16
    AF = mybir.ActivationFunctionType
    ALU = mybir.AluOpType

    nc = bacc.Bacc("TRN2", target_bir_lowering=False, debug=False,
                   num_devices=NCORES)

    search_d = nc.declare_dram_parameter("search", [BPC, 128, 2, 968], bf16, isOutput=False)
    tmpl_d = nc.declare_dram_parameter("tmpl", [128, 2, BPC, 52], bf16, isOutput=False)
    wkt_d = nc.declare_dram_parameter("wkt", [128, 36, 128], bf16, isOutput=False)
    wst_d = nc.declare_dram_parameter("wst", [128, 36, 128], bf16, isOutput=False)
    w1t_d = nc.declare_dram_parameter("w1t", [128, 4, 128], f32r, isOutput=False)
    w2t_d = nc.declare_dram_parameter("w2t", [128, 2, 20], f32r, isOutput=False)
    bnk_d = nc.declare_dram_parameter("bnk", [128, 4], f32, isOutput=False)
    bns_d = nc.declare_dram_parameter("bns", [128, 4], f32, isOutput=False)
    bnh_d = nc.declare_dram_parameter("bnh", [128, 4], f32, isOutput=False)
    b2_d = nc.declare_dram_parameter("b2t", [128, 1], f32, isOutput=False)
    id_d = nc.declare_dram_parameter("ident", [128, 128], f32, isOutput=False)
    out_d = nc.declare_dram_parameter("out", [BPC, 20, 625], f32, isOutput=True)

    def tidx(cig, dy, dx, og):
        return ((cig * 3 + dy) * 3 + dx) * 2 + og

    taps = [(t // 5, t % 5) for t in range(25)]

    with tile.TileContext(nc) as tc:
        with (
            tc.tile_pool(name="wp", bufs=1) as wp,
            tc.tile_pool(name="sp", bufs=2) as sp,
            tc.tile_pool(name="sfp", bufs=3) as sfp,
            tc.tile_pool(name="fp", bufs=4) as fp,
            tc.tile_pool(name="hp", bufs=2) as hp,
            tc.tile_pool(name="pp", bufs=28) as pp,
            tc.tile_pool(name="dp", bufs=96) as dp,
            tc.tile_pool(name="psc", bufs=3, space="PSUM") as psc,
            tc.tile_pool(name="psx", bufs=3, space="PSUM") as psx,
            tc.tile_pool(name="psh", bufs=2, space="PSUM") as psh,
        ):
            wkt = wp.tile([128, 36, 128], bf16)
            wst = wp.tile([128, 36, 128], bf16)
            w1t = wp.tile([128, 4, 128], f32r)
            w2t = wp.tile([128, 2, 20], f32r)
            bnk = wp.tile([128, 4], f32)
            bns = wp.tile([128, 4], f32)
            bnh = wp.tile([128, 4], f32)
            b2t = wp.tile([128, 1], f32)
            ident = wp.tile([128, 128], f32)
            k_in = wp.tile([128, 2, BPC, 52], bf16)
            nc.scalar.dma_start(out=k_in, in_=tmpl_d[:, :, :, :])
            nc.scalar.dma_start(out=bnk, in_=bnk_d[:, :])
            nc.sync.dma_start(out=wkt[:, 0:18, :], in_=wkt_d[:, 0:18, :])
            nc.scalar.dma_start(out=wkt[:, 18:36, :], in_=wkt_d[:, 18:36, :])
            nc.sync.dma_start(out=wst[:, 0:18, :], in_=wst_d[:, 0:18, :])
            nc.scalar.dma_start(out=wst[:, 18:36, :], in_=wst_d[:, 18:36, :])
            nc.gpsimd.dma_start(out=ident, in_=id_d[:, :])
            nc.scalar.dma_start(out=bns, in_=bns_d[:, :])
            nc.sync.dma_start(out=w1t, in_=w1t_d[:, :, :])
            nc.sync.dma_start(out=w2t, in_=w2t_d[:, :, :])
            nc.sync.dma_start(out=bnh, in_=bnh_d[:, :])
            nc.sync.dma_start(out=b2t, in_=b2_d[:, :])

            # conv_kernel branch: all samples batched, exact 5x5 windows
            # rhs 4D AP [128, 8 samples, 5 rows, 5 cols] = 200 columns (bf16)
            k_feat = wp.tile([128, 2, BPC * 36], f32)
            for og in range(2):
                pk = psc.tile([128, 512], f32, tag="conv")
                j = 0
                for cig in range(2):
                    for dy in range(3):
                        for dx in range(3):
                            base = k_in[:, cig, :, :]
                            rhs = bass.AP(
                                tensor=base.tensor,
                                offset=base.offset + dy * 7 + dx,
                                ap=[list(base.ap[0]), [52, BPC], [7, 5], [1, 5]],
                            )
                            nc.tensor.matmul(pk[:, :BPC * 25], wkt[:, tidx(cig, dy, dx, og), :],
                                             rhs, start=(j == 0), stop=(j == 17))
                            j += 1
                # k_feat keeps 36-stride sample blocks; 25 values per sample
                dst = bass.AP(
                    tensor=k_feat.tensor,
                    offset=k_feat[:, og, :].offset,
                    ap=[list(k_feat.ap[0]), [36, BPC], [1, 25]],
                )
                nc.scalar.activation(dst, pk[:, :BPC * 25].rearrange("p (s c) -> p s c", c=25),
                                     AF.Relu, scale=bnk[:, og:og + 1], bias=bnk[:, 2 + og:3 + og])

            def kap(s, og, dy, dx):
                o = s * 36 + dy * 5 + dx
                return k_feat[:, og, o:o + 1]

            state = {}

            def emit_diags(s):
                for og in range(2):
                    p = SCHED[2 * s + og][0]
                    dlist = []
                    for (dy, dx) in taps[:p]:
                        diag = dp.tile([128, 128], bf16, tag="diag")
                        nc.scalar.activation(diag, ident, AF.Copy,
                                             scale=kap(s, og, dy, dx))
                        dlist.append(diag)
                    state[("diag", s, og)] = dlist

            def emit_front(s):
                s_in = state.pop(("sin", s))
                if s + 1 < BPC:
                    nxt = sp.tile([128, 2, 968], bf16, tag="s_in")
                    nc.sync.dma_start(out=nxt, in_=search_d[s + 1, :, :, :])
                    state[("sin", s + 1)] = nxt

                s_feat = sfp.tile([128, 2, 841], bf16, tag="s_feat")
                fv = fp.tile([128, 2, 625], f32, tag="fv")
                featr = fp.tile([128, 2, 640], f32r, tag="featr")
                nc.vector.memset(featr[:, :, 625:640].bitcast(f32), 0.0)
                for og in range(2):
                    # conv_search for this og: 29x29 out, 29-wide rows (bf16)
                    for off, y0c, rws in ((0, 0, 17), (493, 17, 12)):
                        w = rws * 29
                        pc = psc.tile([128, 512], f32, tag="conv")
                        j = 0
                        for cig in range(2):
                            for dy in range(3):
                                for dx in range(3):
                                    rhs = _shift_window(s_in[:, cig, :],
                                                        (y0c + dy) * 31 + dx, rws, 29, 31)
                                    nc.tensor.matmul(pc[:, :w], wst[:, tidx(cig, dy, dx, og), :],
                                                     rhs, start=(j == 0), stop=(j == 17))
                                    j += 1
                        nc.scalar.activation(s_feat[:, og, off:off + w], pc[:, :w], AF.Relu,
                                             scale=bns[:, og:og + 1], bias=bns[:, 2 + og:3 + og])

                    p, qa, qd, d = SCHED[2 * s + og]
                    sf = s_feat[:, og, :]
                    pe_taps = taps[:p]
                    dlist = state.pop(("diag", s, og))

                    # products for Pool adds: ACT share then DVE share
                    prods = []
                    for (dy, dx) in taps[p:p + qa]:
                        prod = pp.tile([128, 625], bf16, tag="prod")
                        win = _shift_window(sf, dy * 29 + dx, 25, 25, 29)
                        nc.scalar.activation(prod, win, AF.Copy,
                                             scale=kap(s, og, dy, dx))
                        prods.append(prod)
                    for (dy, dx) in taps[p + qa:p + qa + qd]:
                        prod = pp.tile([128, 625], bf16, tag="prod")
                        win = _shift_window(sf, dy * 29 + dx, 25, 25, 29)
                        nc.vector.tensor_scalar(prod, win, kap(s, og, dy, dx),
                                                None, ALU.mult)
                        prods.append(prod)

                    # PE xcorr partial -> PSUM -> ACT copy seeds fv (featr if all-PE)
                    fvo = fv[:, og, :]
                    if p > 0:
                        all_pe = (qa + qd + d == 0)
                        dst_acc = featr if all_pe else fv
                        for y0, rows in ((0, 13), (13, 12)):
                            px = psx.tile([128, 325], f32, tag="x")
                            n = rows * 25
                            for i, (dy, dx) in enumerate(pe_taps):
                                rhs = _shift_window(sf, (y0 + dy) * 29 + dx, rows, 25, 29)
                                nc.tensor.matmul(px[:, :n], dlist[i], rhs,
                                                 start=(i == 0), stop=(i == p - 1))
                            nc.scalar.activation(dst_acc[:, og, y0 * 25: y0 * 25 + n],
                                                 px[:, :n], AF.Copy)
                        # Pool accumulates the pre-scaled windows onto fv
                        for prod in prods:
                            nc.gpsimd.tensor_tensor(fvo, fvo, prod, ALU.add)
                    else:
                        # no PE partial: pool seeds fv from the first two products
                        nc.gpsimd.tensor_tensor(fvo, prods[0], prods[1], ALU.add)
                        for prod in prods[2:]:
                            nc.gpsimd.tensor_tensor(fvo, fvo, prod, ALU.add)

                state[s] = (s_feat, fv, featr)

            def emit_stt(s):
                s_feat, fv, featr = state[s]
                for og in range(2):
                    p, qa, qd, d = SCHED[2 * s + og]
                    if d == 0:
                        continue
                    sf = s_feat[:, og, :]
                    fvo = fv[:, og, :]
                    stt_taps = taps[p + qa + qd:]
                    for j, (dy, dx) in enumerate(stt_taps):
                        win = _shift_window(sf, dy * 29 + dx, 25, 25, 29)
                        dst = featr[:, og, 0:625] if j == len(stt_taps) - 1 else fvo
                        nc.vector.scalar_tensor_tensor(dst, win, kap(s, og, dy, dx),
                                                       fvo, ALU.mult, ALU.add)

            def emit_head(s):
                _, _, featr = state.pop(s)
                h = hp.tile([128, 2, 640], f32r, tag="h")
                for og in range(2):
                    for off, w in ((0, 320), (320, 306)):
                        ph = psh.tile([128, 320], f32, tag="ph")
                        nc.tensor.matmul(ph[:, :w], w1t[:, 0 * 2 + og, :],
                                         featr[:, 0, off:off + w],
                                         start=True, stop=False)
                        nc.tensor.matmul(ph[:, :w], w1t[:, 1 * 2 + og, :],
                                         featr[:, 1, off:off + w],
                                         start=False, stop=True)
                        nc.scalar.activation(h[:, og, off:off + w], ph[:, :w], AF.Relu,
                                             scale=bnh[:, og:og + 1], bias=bnh[:, 2 + og:3 + og])

                out_s = hp.tile([128, 640], f32, tag="o")
                for off, w in ((0, 320), (320, 306)):
                    po = psh.tile([128, 320], f32, tag="ph")
                    nc.tensor.matmul(po[0:20, :w], w2t[:, 0, :], h[:, 0, off:off + w],
                                     start=True, stop=False)
                    nc.tensor.matmul(po[0:20, :w], w2t[:, 1, :], h[:, 1, off:off + w],
                                     start=False, stop=True)
                    nc.scalar.activation(out_s[0:20, off:off + w], po[0:20, :w],
                                         AF.Identity, bias=b2t[0:20, 0:1])
                nc.sync.dma_start(out=out_d[s, :, :], in_=out_s[0:20, 0:625])

            def emit_all():
                sin0 = sp.tile([128, 2, 968], bf16, tag="s_in")
                nc.scalar.dma_start(out=sin0, in_=search_d[0, :, :, :])
                state[("sin", 0)] = sin0
                emit_diags(0)
                emit_diags(1)
                for s in range(BPC):
                    emit_front(s)
                    if s + 2 < BPC:
                        emit_diags(s + 2)
                    if s >= 1:
                        emit_stt(s - 1)
                    if s >= 3:
                        emit_head(s - 3)
                emit_stt(BPC - 1)
                emit_head(BPC - 3)
                emit_head(BPC - 2)
                emit_head(BPC - 1)

            if bench_R:
                with tc.For_i(0, bench_R, 1,
                              hint_engines=(mybir.EngineType.PE,
                                            mybir.EngineType.DVE,
                                            mybir.EngineType.Activation)):
                    emit_all()
            else:
                emit_all()

    nc.compile()
    return nc


def _pack(inputs):
    import ml_dtypes
    f32 = np.float32
    bf16 = ml_dtypes.bfloat16
    kern = np.ascontiguousarray(inputs["kernel"], dtype=f32)
    search = np.ascontiguousarray(inputs["search"], dtype=f32)
    wk, ws = inputs["wk"].astype(f32), inputs["ws"].astype(f32)
    w1, w2, b2 = inputs["w1"].astype(f32), inputs["w2"].astype(f32), inputs["b2"].astype(f32)

    def fold(scale, bias, mean, var):
        inv = scale.astype(f32) / np.sqrt(var.astype(f32) + EPS)
        sh = bias.astype(f32) - mean.astype(f32) * inv
        arr = np.zeros((128, 4), f32)
        arr[:, 0:2] = inv.reshape(2, 128).T
        arr[:, 2:4] = sh.reshape(2, 128).T
        return arr

    bnk = fold(inputs["bnk_scale"], inputs["bnk_bias"], inputs["bnk_mean"], inputs["bnk_var"])
    bns = fold(inputs["bns_scale"], inputs["bns_bias"], inputs["bns_mean"], inputs["bns_var"])
    bnh = fold(inputs["bnh_scale"], inputs["bnh_bias"], inputs["bnh_mean"], inputs["bnh_var"])

    # conv weights -> lhsT tiles [ci, (cig,dy,dx,og), co], bf16
    def conv_w(w):
        w6 = w.reshape(2, 128, 2, 128, 3, 3)           # og co cig ci dy dx
        return np.ascontiguousarray(
            w6.transpose(3, 2, 4, 5, 0, 1).reshape(128, 36, 128).astype(bf16))

    wkt, wst = conv_w(wk), conv_w(ws)
    w1t = np.ascontiguousarray(
        w1[:, :, 0, 0].reshape(2, 128, 2, 128).transpose(3, 2, 0, 1).reshape(128, 4, 128))
    w2t = np.ascontiguousarray(
        w2[:, :, 0, 0].reshape(20, 2, 128).transpose(2, 1, 0))
    b2t = np.zeros((128, 1), f32)
    b2t[:20, 0] = b2
    ident = np.eye(128, dtype=f32)

    # search [64,256,31,31] -> per core [8, 128(ci), 2(cig), 961] bf16
    sr = np.zeros((NCORES, BPC, 128, 2, 968), bf16)
    sr[..., :961] = search.reshape(NCORES, BPC, 2, 128, 961).transpose(0, 1, 3, 2, 4).astype(bf16)
    # kernel [64,256,7,7] -> per core [128(ci), 2(cig), 8(s), 52] bf16
    kr = np.zeros((NCORES, 128, 2, BPC, 52), bf16)
    kr[..., :49] = kern.reshape(NCORES, BPC, 2, 128, 49).transpose(0, 3, 2, 1, 4).astype(bf16)

    in_maps = []
    for c in range(NCORES):
        in_maps.append({
            "search": np.ascontiguousarray(sr[c]),
            "tmpl": np.ascontiguousarray(kr[c]),
            "wkt": wkt, "wst": wst, "w1t": w1t, "w2t": w2t,
            "bnk": bnk, "bns": bns, "bnh": bnh, "b2t": b2t, "ident": ident,
        })
    return in_maps


def get_program(bench_R=0):
    key = f"nc{bench_R}"
    if key not in _CACHE:
        _CACHE[key] = _build(bench_R)
    return _CACHE[key]


def kernel(**inputs):
    from concourse.bass_utils import run_bass_kernel_spmd
    nc = get_program()
    in_maps = _pack(inputs)
    res = run_bass_kernel_spmd(nc, in_maps, core_ids=list(range(NCORES)))
    out = np.stack([res.results[c]["out"] for c in range(NCORES)], axis=0)
    return out.reshape(64, 20, 25, 25).astype(np.float32)


# revision 14
# speedup vs baseline: 1.1038x; 1.1038x over previous
"""Trainium2 Bass kernel for nn_DepthwiseXCorr (SiamRPN++-style depthwise-xcorr head).

Pipeline per sample (data-parallel over batch: 64 samples -> 8 cores x 8):
  conv3x3(kernel,wk)+BN+ReLU -> k_feat [256,5,5]
  conv3x3(search,ws)+BN+ReLU -> s_feat [256,29,29]   (bf16, 29-wide rows)
  depthwise xcorr(s_feat,k_feat) -> feat [256,25,25]
  1x1 conv w1 + BN + ReLU -> h [256,25,25]
  1x1 conv w2 + b2 -> out [20,25,25]

Convs run in bf16 (no fp32r even-size restriction -> no garbage columns);
the head 1x1 convs run in fp32r. The depthwise xcorr is split across
engines per (sample, og) chain of 25 taps:
  - PE: per-tap diagonal-weight matmuls (bf16 diags built on ACT, two
    samples ahead so PE never waits)
  - Pool (GPSIMD): tensor_tensor adds of pre-scaled windows (products from
    ACT scaled-copies or DVE fast tensor_scalar)
  - DVE: scalar_tensor_tensor multiply-accumulate taps
Partial sums chain through one f32 accumulator (ACT copies the PE PSUM
partial in, Pool then DVE accumulate on top); the final tap writes the
f32r head input. Emission is software-pipelined: front(s), stt(s-1),
head(s-3); the last sample is all-PE so the drain is short.
"""
import numpy as np

EPS = 1e-5
NCORES = 8
BPC = 8          # samples per core

# per og-sample (idx = 2*s + og): (p, qa, qd, d)
#   p taps on PE, qa products on ACT -> Pool adds, qd products on DVE ->
#   Pool adds, d taps on DVE STT. p+qa+qd+d = 25.
SCHED = [
    (5, 4, 2, 14), (5, 4, 2, 14),
    (5, 4, 2, 14), (5, 4, 2, 14),
    (5, 4, 2, 14), (5, 4, 2, 14),
    (5, 4, 2, 14), (5, 4, 2, 14),
    (6, 3, 3, 13), (6, 3, 3, 13),
    (8, 4, 2, 11), (8, 4, 2, 11),
    (10, 4, 2, 9), (11, 4, 2, 8),
    (16, 5, 1, 3), (25, 0, 0, 0),
]

_CACHE = {}


def _shift_window(ap_2d, base_off, rows, cols, rowstride):
    """AP reading [128, rows, cols] window at element offset base_off of a
    [128, W] SBUF view, row stride in elements."""
    import concourse.bass as bass
    return bass.AP(
        tensor=ap_2d.tensor,
        offset=ap_2d.offset + base_off,
        ap=[list(ap_2d.ap[0]), [rowstride, rows], [1, cols]],
    )


def _build(bench_R=0):
    import concourse.bacc as bacc
    import concourse.bass as bass
    import concourse.mybir as mybir
    import concourse.tile as tile

    f32 = mybir.dt.float32
    f32r = mybir.dt.float32r
    bf16 = mybir.dt.bfloat16
    AF = mybir.ActivationFunctionType
    ALU = mybir.AluOpType

    nc = bacc.Bacc("TRN2", target_bir_lowering=False, debug=False,
                   num_devices=NCORES)

    search_d = nc.declare_dram_parameter("search", [BPC, 128, 2, 968], bf16, isOutput=False)
    tmpl_d = nc.declare_dram_parameter("tmpl", [128, 2, BPC, 52], bf16, isOutput=False)
    wkt_d = nc.declare_dram_parameter("wkt", [128, 36, 128], bf16, isOutput=False)
    wst_d = nc.declare_dram_parameter("wst", [128, 36, 128], bf16, isOutput=False)
    w1t_d = nc.declare_dram_parameter("w1t", [128, 4, 128], f32r, isOutput=False)
    w2t_d = nc.declare_dram_parameter("w2t", [128, 2, 20], f32r, isOutput=False)
    bnk_d = nc.declare_dram_parameter("bnk", [128, 4], f32, isOutput=False)
    bns_d = nc.declare_dram_parameter("bns", [128, 4], f32, isOutput=False)
    bnh_d = nc.declare_dram_parameter("bnh", [128, 4], f32, isOutput=False)
    b2_d = nc.declare_dram_parameter("b2t", [128, 1], f32, isOutput=False)
    id_d = nc.declare_dram_parameter("ident", [128, 128], f32, isOutput=False)
    out_d = nc.declare_dram_parameter("out", [BPC, 20, 625], f32, isOutput=True)

    def tidx(cig, dy, dx, og):
        return ((cig * 3 + dy) * 3 + dx) * 2 + og

    taps = [(t // 5, t % 5) for t in range(25)]

    with tile.TileContext(nc) as tc:
        with (
            tc.tile_pool(name="wp", bufs=1) as wp,
            tc.tile_pool(name="sp", bufs=2) as sp,
            tc.tile_pool(name="sfp", bufs=3) as sfp,
            tc.tile_pool(name="fp", bufs=4) as fp,
            tc.tile_pool(name="hp", bufs=2) as hp,
            tc.tile_pool(name="pp", bufs=28) as pp,
            tc.tile_pool(name="dp", bufs=96) as dp,
            tc.tile_pool(name="psc", bufs=3, space="PSUM") as psc,
            tc.tile_pool(name="psx", bufs=3, space="PSUM") as psx,
            tc.tile_pool(name="psh", bufs=2, space="PSUM") as psh,
        ):
            wkt = wp.tile([128, 36, 128], bf16)
            wst = wp.tile([128, 36, 128], bf16)
            w1t = wp.tile([128, 4, 128], f32r)
            w2t = wp.tile([128, 2, 20], f32r)
            bnk = wp.tile([128, 4], f32)
            bns = wp.tile([128, 4], f32)
            bnh = wp.tile([128, 4], f32)
            b2t = wp.tile([128, 1], f32)
            ident = wp.tile([128, 128], f32)
            k_in = wp.tile([128, 2, BPC, 52], bf16)
            nc.scalar.dma_start(out=k_in, in_=tmpl_d[:, :, :, :])
            nc.scalar.dma_start(out=bnk, in_=bnk_d[:, :])
            nc.sync.dma_start(out=wkt[:, 0:18, :], in_=wkt_d[:, 0:18, :])
            nc.scalar.dma_start(out=wkt[:, 18:36, :], in_=wkt_d[:, 18:36, :])
            nc.sync.dma_start(out=wst[:, 0:18, :], in_=wst_d[:, 0:18, :])
            nc.scalar.dma_start(out=wst[:, 18:36, :], in_=wst_d[:, 18:36, :])
            nc.gpsimd.dma_start(out=ident, in_=id_d[:, :])
            nc.scalar.dma_start(out=bns, in_=bns_d[:, :])
            nc.sync.dma_start(out=w1t, in_=w1t_d[:, :, :])
            nc.sync.dma_start(out=w2t, in_=w2t_d[:, :, :])
            nc.sync.dma_start(out=bnh, in_=bnh_d[:, :])
            nc.sync.dma_start(out=b2t, in_=b2_d[:, :])

            # conv_kernel branch: all samples batched, exact 5x5 windows
            # rhs 4D AP [128, 8 samples, 5 rows, 5 cols] = 200 columns (bf16)
            k_feat = wp.tile([128, 2, BPC * 36], f32)
            for og in range(2):
                pk = psc.tile([128, 512], f32, tag="conv")
                j = 0
                for cig in range(2):
                    for dy in range(3):
                        for dx in range(3):
                            base = k_in[:, cig, :, :]
                            rhs = bass.AP(
                                tensor=base.tensor,
                                offset=base.offset + dy * 7 + dx,
                                ap=[list(base.ap[0]), [52, BPC], [7, 5], [1, 5]],
                            )
                            nc.tensor.matmul(pk[:, :BPC * 25], wkt[:, tidx(cig, dy, dx, og), :],
                                             rhs, start=(j == 0), stop=(j == 17))
                            j += 1
                # k_feat keeps 36-stride sample blocks; 25 values per sample
                dst = bass.AP(
                    tensor=k_feat.tensor,
                    offset=k_feat[:, og, :].offset,
                    ap=[list(k_feat.ap[0]), [36, BPC], [1, 25]],
                )
                nc.scalar.activation(dst, pk[:, :BPC * 25].rearrange("p (s c) -> p s c", c=25),
                                     AF.Relu, scale=bnk[:, og:og + 1], bias=bnk[:, 2 + og:3 + og])

            def kap(s, og, dy, dx):
                o = s * 36 + dy * 5 + dx
                return k_feat[:, og, o:o + 1]

            state = {}

            def emit_diags(s):
                for og in range(2):
                    p = SCHED[2 * s + og][0]
                    dlist = []
                    for (dy, dx) in taps[:p]:
                        diag = dp.tile([128, 128], bf16, tag="diag")
                        nc.scalar.activation(diag, ident, AF.Copy,
                                             scale=kap(s, og, dy, dx))
                        dlist.append(diag)
                    state[("diag", s, og)] = dlist

            def emit_front(s):
                s_in = state.pop(("sin", s))
                if s + 1 < BPC:
                    nxt = sp.tile([128, 2, 968], bf16, tag="s_in")
                    nc.sync.dma_start(out=nxt, in_=search_d[s + 1, :, :, :])
                    state[("sin", s + 1)] = nxt

                s_feat = sfp.tile([128, 2, 841], bf16, tag="s_feat")
                fv = fp.tile([128, 2, 625], f32, tag="fv")
                featr = fp.tile([128, 2, 640], f32r, tag="featr")
                nc.vector.memset(featr[:, :, 625:640].bitcast(f32), 0.0)
                for og in range(2):
                    # conv_search for this og: 29x29 out, 29-wide rows (bf16)
                    for off, y0c, rws in ((0, 0, 17), (493, 17, 12)):
                        w = rws * 29
                        pc = psc.tile([128, 512], f32, tag="conv")
                        j = 0
                        for cig in range(2):
                            for dy in range(3):
                                for dx in range(3):
                                    rhs = _shift_window(s_in[:, cig, :],
                                                        (y0c + dy) * 31 + dx, rws, 29, 31)
                                    nc.tensor.matmul(pc[:, :w], wst[:, tidx(cig, dy, dx, og), :],
                                                     rhs, start=(j == 0), stop=(j == 17))
                                    j += 1
                        nc.scalar.activation(s_feat[:, og, off:off + w], pc[:, :w], AF.Relu,
                                             scale=bns[:, og:og + 1], bias=bns[:, 2 + og:3 + og])

                    p, qa, qd, d = SCHED[2 * s + og]
                    sf = s_feat[:, og, :]
                    pe_taps = taps[:p]
                    dlist = state.pop(("diag", s, og))

                    # products for Pool adds: ACT share then DVE share
                    prods = []
                    for (dy, dx) in taps[p:p + qa]:
                        prod = pp.tile([128, 625], bf16, tag="prod")
                        win = _shift_window(sf, dy * 29 + dx, 25, 25, 29)
                        nc.scalar.activation(prod, win, AF.Copy,
                                             scale=kap(s, og, dy, dx))
                        prods.append(prod)
                    for (dy, dx) in taps[p + qa:p + qa + qd]:
                        prod = pp.tile([128, 625], bf16, tag="prod")
                        win = _shift_window(sf, dy * 29 + dx, 25, 25, 29)
                        nc.vector.tensor_scalar(prod, win, kap(s, og, dy, dx),
                                                None, ALU.mult)
                        prods.append(prod)

                    # PE xcorr partial -> PSUM -> ACT copy seeds fv (featr if all-PE)
                    fvo = fv[:, og, :]
                    if p > 0:
                        all_pe = (qa + qd + d == 0)
                        dst_acc = featr if all_pe else fv
                        for y0, rows in ((0, 13), (13, 12)):
                            px = psx.tile([128, 325], f32, tag="x")
                            n = rows * 25
                            for i, (dy, dx) in enumerate(pe_taps):
                                rhs = _shift_window(sf, (y0 + dy) * 29 + dx, rows, 25, 29)
                                nc.tensor.matmul(px[:, :n], dlist[i], rhs,
                                                 start=(i == 0), stop=(i == p - 1))
                            nc.scalar.activation(dst_acc[:, og, y0 * 25: y0 * 25 + n],
                                                 px[:, :n], AF.Copy)
                        # Pool accumulates the pre-scaled windows onto fv
                        for prod in prods:
                            nc.gpsimd.tensor_tensor(fvo, fvo, prod, ALU.add)
                    else:
                        # no PE partial: pool seeds fv from the first two products
                        nc.gpsimd.tensor_tensor(fvo, prods[0], prods[1], ALU.add)
                        for prod in prods[2:]:
                            nc.gpsimd.tensor_tensor(fvo, fvo, prod, ALU.add)

                state[s] = (s_feat, fv, featr)

            def emit_stt(s):
                s_feat, fv, featr = state[s]
                for og in range(2):
                    p, qa, qd, d = SCHED[2 * s + og]
                    if d == 0:
                        continue
                    sf = s_feat[:, og, :]
                    fvo = fv[:, og, :]
                    stt_taps = taps[p + qa + qd:]
                    for j, (dy, dx) in enumerate(stt_taps):
                        win = _shift_window(sf, dy * 29 + dx, 25, 25, 29)
                        dst = featr[:, og, 0:625] if j == len(stt_taps) - 1 else fvo
                        nc.vector.scalar_tensor_tensor(dst, win, kap(s, og, dy, dx),
                                                       fvo, ALU.mult, ALU.add)

            def emit_head(s):
                _, _, featr = state.pop(s)
                h = hp.tile([128, 2, 640], f32r, tag="h")
                for og in range(2):
                    for off, w in ((0, 320), (320, 306)):
                        ph = psh.tile([128, 320], f32, tag="ph")
                        nc.tensor.matmul(ph[:, :w], w1t[:, 0 * 2 + og, :],
                                         featr[:, 0, off:off + w],
                                         start=True, stop=False)
                        nc.tensor.matmul(ph[:, :w], w1t[:, 1 * 2 + og, :],
                                         featr[:, 1, off:off + w],
                                         start=False, stop=True)
                        nc.scalar.activation(h[:, og, off:off + w], ph[:, :w], AF.Relu,
                                             scale=bnh[:, og:og + 1], bias=bnh[:, 2 + og:3 + og])

                out_s = hp.tile([128, 640], f32, tag="o")
                for off, w in ((0, 320), (320, 306)):
                    po = psh.tile([128, 320], f32, tag="ph")
                    nc.tensor.matmul(po[0:20, :w], w2t[:, 0, :], h[:, 0, off:off + w],
                                     start=True, stop=False)
                    nc.tensor.matmul(po[0:20, :w], w2t[:, 1, :], h[:, 1, off:off + w],
                                     start=False, stop=True)
                    nc.scalar.activation(out_s[0:20, off:off + w], po[0:20, :w],
                                         AF.Identity, bias=b2t[0:20, 0:1])
                nc.sync.dma_start(out=out_d[s, :, :], in_=out_s[0:20, 0:625])

            def emit_all():
                sin0 = sp.tile([128, 2, 968], bf16, tag="s_in")
                nc.scalar.dma_start(out=sin0, in_=search_d[0, :, :, :])
                state[("sin", 0)] = sin0
                emit_diags(0)
                emit_diags(1)
                for s in range(BPC):
                    emit_front(s)
                    if s + 2 < BPC:
                        emit_diags(s + 2)
                    if s >= 1:
                        emit_stt(s - 1)
                    if s >= 3:
                        emit_head(s - 3)
                emit_stt(BPC - 1)
                emit_head(BPC - 3)
                emit_head(BPC - 2)
                emit_head(BPC - 1)

            if bench_R:
                with tc.For_i(0, bench_R, 1,
                              hint_engines=(mybir.EngineType.PE,
                                            mybir.EngineType.DVE,
                                            mybir.EngineType.Activation)):
                    emit_all()
            else:
                emit_all()

    nc.compile()
    return nc


def _pack(inputs):
    import ml_dtypes
    f32 = np.float32
    bf16 = ml_dtypes.bfloat16
    kern = np.ascontiguousarray(inputs["kernel"], dtype=f32)
    search = np.ascontiguousarray(inputs["search"], dtype=f32)
    wk, ws = inputs["wk"].astype(f32), inputs["ws"].astype(f32)
    w1, w2, b2 = inputs["w1"].astype(f32), inputs["w2"].astype(f32), inputs["b2"].astype(f32)

    def fold(scale, bias, mean, var):
        inv = scale.astype(f32) / np.sqrt(var.astype(f32) + EPS)
        sh = bias.astype(f32) - mean.astype(f32) * inv
        arr = np.zeros((128, 4), f32)
        arr[:, 0:2] = inv.reshape(2, 128).T
        arr[:, 2:4] = sh.reshape(2, 128).T
        return arr

    bnk = fold(inputs["bnk_scale"], inputs["bnk_bias"], inputs["bnk_mean"], inputs["bnk_var"])
    bns = fold(inputs["bns_scale"], inputs["bns_bias"], inputs["bns_mean"], inputs["bns_var"])
    bnh = fold(inputs["bnh_scale"], inputs["bnh_bias"], inputs["bnh_mean"], inputs["bnh_var"])

    # conv weights -> lhsT tiles [ci, (cig,dy,dx,og), co], bf16
    def conv_w(w):
        w6 = w.reshape(2, 128, 2, 128, 3, 3)           # og co cig ci dy dx
        return np.ascontiguousarray(
            w6.transpose(3, 2, 4, 5, 0, 1).reshape(128, 36, 128).astype(bf16))

    wkt, wst = conv_w(wk), conv_w(ws)
    w1t = np.ascontiguousarray(
        w1[:, :, 0, 0].reshape(2, 128, 2, 128).transpose(3, 2, 0, 1).reshape(128, 4, 128))
    w2t = np.ascontiguousarray(
        w2[:, :, 0, 0].reshape(20, 2, 128).transpose(2, 1, 0))
    b2t = np.zeros((128, 1), f32)
    b2t[:20, 0] = b2
    ident = np.eye(128, dtype=f32)

    # search [64,256,31,31] -> per core [8, 128(ci), 2(cig), 961] bf16
    sr = np.zeros((NCORES, BPC, 128, 2, 968), bf16)
    sr[..., :961] = search.reshape(NCORES, BPC, 2, 128, 961).transpose(0, 1, 3, 2, 4).astype(bf16)
    # kernel [64,256,7,7] -> per core [128(ci), 2(cig), 8(s), 52] bf16
    kr = np.zeros((NCORES, 128, 2, BPC, 52), bf16)
    kr[..., :49] = kern.reshape(NCORES, BPC, 2, 128, 49).transpose(0, 3, 2, 1, 4).astype(bf16)

    in_maps = []
    for c in range(NCORES):
        in_maps.append({
            "search": np.ascontiguousarray(sr[c]),
            "tmpl": np.ascontiguousarray(kr[c]),
            "wkt": wkt, "wst": wst, "w1t": w1t, "w2t": w2t,
            "bnk": bnk, "bns": bns, "bnh": bnh, "b2t": b2t, "ident": ident,
        })
    return in_maps


def get_program(bench_R=0):
    key = f"nc{bench_R}"
    if key not in _CACHE:
        _CACHE[key] = _build(bench_R)
    return _CACHE[key]


def kernel(**inputs):
    from concourse.bass_utils import run_bass_kernel_spmd
    nc = get_program()
    in_maps = _pack(inputs)
    res = run_bass_kernel_spmd(nc, in_maps, core_ids=list(range(NCORES)))
    out = np.stack([res.results[c]["out"] for c in range(NCORES)], axis=0)
    return out.reshape(64, 20, 25, 25).astype(np.float32)


# revision 15
# speedup vs baseline: 1.1302x; 1.0240x over previous
"""Trainium2 Bass kernel for nn_DepthwiseXCorr (SiamRPN++-style depthwise-xcorr head).

Pipeline per sample (data-parallel over batch: 64 samples -> 8 cores x 8):
  conv3x3(kernel,wk)+BN+ReLU -> k_feat [256,5,5]
  conv3x3(search,ws)+BN+ReLU -> s_feat [256,29,29]   (bf16, 29-wide rows)
  depthwise xcorr(s_feat,k_feat) -> feat [256,25,25]
  1x1 conv w1 + BN + ReLU -> h [256,25,25]
  1x1 conv w2 + b2 -> out [20,25,25]

Convs run in bf16 (no fp32r even-size restriction -> no garbage columns);
the head 1x1 convs run in fp32r. The depthwise xcorr is split across
engines per (sample, og) chain of 25 taps:
  - PE: per-tap diagonal-weight matmuls (bf16 diags built on ACT, two
    samples ahead so PE never waits)
  - Pool (GPSIMD): tensor_tensor adds of pre-scaled windows (products from
    ACT scaled-copies or DVE fast tensor_scalar)
  - DVE: scalar_tensor_tensor multiply-accumulate taps
Partial sums chain through one f32 accumulator (ACT copies the PE PSUM
partial in, Pool then DVE accumulate on top); the final tap writes the
f32r head input. Emission is software-pipelined: front(s), stt(s-1),
head(s-3); the last sample is all-PE so the drain is short.
"""
import numpy as np

EPS = 1e-5
NCORES = 8
BPC = 8          # samples per core

# per og-sample (idx = 2*s + og): (p, qa, qd, d)
#   p taps on PE, qa products on ACT -> Pool adds, qd products on DVE ->
#   Pool adds, d taps on DVE STT. p+qa+qd+d = 25.
SCHED = [
    (5, 4, 2, 14), (5, 4, 2, 14),
    (5, 4, 2, 14), (5, 4, 2, 14),
    (5, 4, 2, 14), (5, 4, 2, 14),
    (5, 4, 2, 14), (5, 4, 2, 14),
    (6, 3, 3, 13), (6, 3, 3, 13),
    (8, 4, 2, 11), (8, 4, 2, 11),
    (10, 4, 2, 9), (11, 4, 2, 8),
    (16, 5, 1, 3), (25, 0, 0, 0),
]

_CACHE = {}


def _shift_window(ap_2d, base_off, rows, cols, rowstride):
    """AP reading [128, rows, cols] window at element offset base_off of a
    [128, W] SBUF view, row stride in elements."""
    import concourse.bass as bass
    return bass.AP(
        tensor=ap_2d.tensor,
        offset=ap_2d.offset + base_off,
        ap=[list(ap_2d.ap[0]), [rowstride, rows], [1, cols]],
    )


def _build(bench_R=0):
    import concourse.bacc as bacc
    import concourse.bass as bass
    import concourse.mybir as mybir
    import concourse.tile as tile

    f32 = mybir.dt.float32
    f32r = mybir.dt.float32r
    bf16 = mybir.dt.bfloat16
    AF = mybir.ActivationFunctionType
    ALU = mybir.AluOpType

    nc = bacc.Bacc("TRN2", target_bir_lowering=False, debug=False,
                   num_devices=NCORES)

    search_d = nc.declare_dram_parameter("search", [BPC, 128, 2, 968], bf16, isOutput=False)
    tmpl_d = nc.declare_dram_parameter("tmpl", [128, 2, BPC, 52], bf16, isOutput=False)
    wkt_d = nc.declare_dram_parameter("wkt", [128, 36, 128], bf16, isOutput=False)
    wst_d = nc.declare_dram_parameter("wst", [128, 36, 128], bf16, isOutput=False)
    w1t_d = nc.declare_dram_parameter("w1t", [128, 4, 128], f32r, isOutput=False)
    w2t_d = nc.declare_dram_parameter("w2t", [128, 2, 20], f32r, isOutput=False)
    bnk_d = nc.declare_dram_parameter("bnk", [128, 4], f32, isOutput=False)
    bns_d = nc.declare_dram_parameter("bns", [128, 4], f32, isOutput=False)
    bnh_d = nc.declare_dram_parameter("bnh", [128, 4], f32, isOutput=False)
    b2_d = nc.declare_dram_parameter("b2t", [128, 1], f32, isOutput=False)
    id_d = nc.declare_dram_parameter("ident", [128, 128], f32, isOutput=False)
    out_d = nc.declare_dram_parameter("out", [BPC, 20, 625], f32, isOutput=True)

    def tidx(cig, dy, dx, og):
        return ((cig * 3 + dy) * 3 + dx) * 2 + og

    taps = [(t // 5, t % 5) for t in range(25)]

    with tile.TileContext(nc) as tc:
        with (
            tc.tile_pool(name="wp", bufs=1) as wp,
            tc.tile_pool(name="sp", bufs=2) as sp,
            tc.tile_pool(name="sfp", bufs=3) as sfp,
            tc.tile_pool(name="fp", bufs=4) as fp,
            tc.tile_pool(name="hp", bufs=2) as hp,
            tc.tile_pool(name="pp", bufs=28) as pp,
            tc.tile_pool(name="dp", bufs=96) as dp,
            tc.tile_pool(name="psc", bufs=3, space="PSUM") as psc,
            tc.tile_pool(name="psx", bufs=3, space="PSUM") as psx,
            tc.tile_pool(name="psh", bufs=2, space="PSUM") as psh,
        ):
            wkt = wp.tile([128, 36, 128], bf16)
            wst = wp.tile([128, 36, 128], bf16)
            w1t = wp.tile([128, 4, 128], f32r)
            w2t = wp.tile([128, 2, 20], f32r)
            bnk = wp.tile([128, 4], f32)
            bns = wp.tile([128, 4], f32)
            bnh = wp.tile([128, 4], f32)
            b2t = wp.tile([128, 1], f32)
            ident = wp.tile([128, 128], f32)
            k_in = wp.tile([128, 2, BPC, 52], bf16)
            nc.scalar.dma_start(out=k_in, in_=tmpl_d[:, :, :, :])
            nc.scalar.dma_start(out=bnk, in_=bnk_d[:, :])
            nc.sync.dma_start(out=wkt[:, 0:18, :], in_=wkt_d[:, 0:18, :])
            nc.scalar.dma_start(out=wkt[:, 18:36, :], in_=wkt_d[:, 18:36, :])
            nc.sync.dma_start(out=wst[:, 0:18, :], in_=wst_d[:, 0:18, :])
            nc.scalar.dma_start(out=wst[:, 18:36, :], in_=wst_d[:, 18:36, :])
            nc.gpsimd.dma_start(out=ident, in_=id_d[:, :])
            nc.scalar.dma_start(out=bns, in_=bns_d[:, :])
            nc.sync.dma_start(out=w1t, in_=w1t_d[:, :, :])
            nc.sync.dma_start(out=w2t, in_=w2t_d[:, :, :])
            nc.sync.dma_start(out=bnh, in_=bnh_d[:, :])
            nc.sync.dma_start(out=b2t, in_=b2_d[:, :])

            # conv_kernel branch: all samples batched, exact 5x5 windows
            # rhs 4D AP [128, 8 samples, 5 rows, 5 cols] = 200 columns (bf16)
            k_feat = wp.tile([128, 2, BPC * 36], f32)
            for og in range(2):
                pk = psc.tile([128, 512], f32, tag="conv")
                j = 0
                for cig in range(2):
                    for dy in range(3):
                        for dx in range(3):
                            base = k_in[:, cig, :, :]
                            rhs = bass.AP(
                                tensor=base.tensor,
                                offset=base.offset + dy * 7 + dx,
                                ap=[list(base.ap[0]), [52, BPC], [7, 5], [1, 5]],
                            )
                            nc.tensor.matmul(pk[:, :BPC * 25], wkt[:, tidx(cig, dy, dx, og), :],
                                             rhs, start=(j == 0), stop=(j == 17))
                            j += 1
                # k_feat keeps 36-stride sample blocks; 25 values per sample
                dst = bass.AP(
                    tensor=k_feat.tensor,
                    offset=k_feat[:, og, :].offset,
                    ap=[list(k_feat.ap[0]), [36, BPC], [1, 25]],
                )
                nc.scalar.activation(dst, pk[:, :BPC * 25].rearrange("p (s c) -> p s c", c=25),
                                     AF.Relu, scale=bnk[:, og:og + 1], bias=bnk[:, 2 + og:3 + og])

            def kap(s, og, dy, dx):
                o = s * 36 + dy * 5 + dx
                return k_feat[:, og, o:o + 1]

            state = {}

            def emit_diags(s):
                for og in range(2):
                    p = SCHED[2 * s + og][0]
                    dlist = []
                    for (dy, dx) in taps[:p]:
                        diag = dp.tile([128, 128], bf16, tag="diag")
                        nc.scalar.activation(diag, ident, AF.Copy,
                                             scale=kap(s, og, dy, dx))
                        dlist.append(diag)
                    state[("diag", s, og)] = dlist

            def emit_front(s):
                s_in = state.pop(("sin", s))
                if s + 1 < BPC:
                    nxt = sp.tile([128, 2, 968], bf16, tag="s_in")
                    nc.sync.dma_start(out=nxt, in_=search_d[s + 1, :, :, :])
                    state[("sin", s + 1)] = nxt

                s_feat = sfp.tile([128, 2, 841], bf16, tag="s_feat")
                fv = fp.tile([128, 2, 625], f32, tag="fv")
                featr = fp.tile([128, 2, 640], f32r, tag="featr")
                nc.vector.memset(featr[:, :, 625:640].bitcast(f32), 0.0)
                for og in range(2):
                    p, qa, qd, d = SCHED[2 * s + og]
                    sf = s_feat[:, og, :]
                    pe_taps = taps[:p]
                    dlist = state.pop(("diag", s, og))
                    all_pe = (qa + qd + d == 0)
                    dst_acc = featr if all_pe else fv
                    fvo = fv[:, og, :]

                    # conv chunk A -> xcorr chunk A -> conv chunk B -> xcorr B
                    # (xcorr rows 0-12 only need s_feat rows 0-16 = chunk A)
                    for (off, y0c, rws), (y0, rows) in (((0, 0, 17), (0, 13)),
                                                        ((493, 17, 12), (13, 12))):
                        w = rws * 29
                        pc = psc.tile([128, 512], f32, tag="conv")
                        j = 0
                        for cig in range(2):
                            for dy in range(3):
                                for dx in range(3):
                                    rhs = _shift_window(s_in[:, cig, :],
                                                        (y0c + dy) * 31 + dx, rws, 29, 31)
                                    nc.tensor.matmul(pc[:, :w], wst[:, tidx(cig, dy, dx, og), :],
                                                     rhs, start=(j == 0), stop=(j == 17))
                                    j += 1
                        nc.scalar.activation(s_feat[:, og, off:off + w], pc[:, :w], AF.Relu,
                                             scale=bns[:, og:og + 1], bias=bns[:, 2 + og:3 + og])
                        if p > 0:
                            px = psx.tile([128, 325], f32, tag="x")
                            n = rows * 25
                            for i, (dy, dx) in enumerate(pe_taps):
                                rhs = _shift_window(sf, (y0 + dy) * 29 + dx, rows, 25, 29)
                                nc.tensor.matmul(px[:, :n], dlist[i], rhs,
                                                 start=(i == 0), stop=(i == p - 1))
                            nc.scalar.activation(dst_acc[:, og, y0 * 25: y0 * 25 + n],
                                                 px[:, :n], AF.Copy)

                    # products for Pool adds: ACT share then DVE share
                    prods = []
                    for (dy, dx) in taps[p:p + qa]:
                        prod = pp.tile([128, 625], bf16, tag="prod")
                        win = _shift_window(sf, dy * 29 + dx, 25, 25, 29)
                        nc.scalar.activation(prod, win, AF.Copy,
                                             scale=kap(s, og, dy, dx))
                        prods.append(prod)
                    for (dy, dx) in taps[p + qa:p + qa + qd]:
                        prod = pp.tile([128, 625], bf16, tag="prod")
                        win = _shift_window(sf, dy * 29 + dx, 25, 25, 29)
                        nc.vector.tensor_scalar(prod, win, kap(s, og, dy, dx),
                                                None, ALU.mult)
                        prods.append(prod)

                    # Pool accumulates the pre-scaled windows onto fv
                    if p > 0:
                        for prod in prods:
                            nc.gpsimd.tensor_tensor(fvo, fvo, prod, ALU.add)
                    else:
                        nc.gpsimd.tensor_tensor(fvo, prods[0], prods[1], ALU.add)
                        for prod in prods[2:]:
                            nc.gpsimd.tensor_tensor(fvo, fvo, prod, ALU.add)

                state[s] = (s_feat, fv, featr)

            def emit_stt(s):
                s_feat, fv, featr = state[s]
                for og in range(2):
                    p, qa, qd, d = SCHED[2 * s + og]
                    if d == 0:
                        continue
                    sf = s_feat[:, og, :]
                    fvo = fv[:, og, :]
                    stt_taps = taps[p + qa + qd:]
                    for j, (dy, dx) in enumerate(stt_taps):
                        win = _shift_window(sf, dy * 29 + dx, 25, 25, 29)
                        dst = featr[:, og, 0:625] if j == len(stt_taps) - 1 else fvo
                        nc.vector.scalar_tensor_tensor(dst, win, kap(s, og, dy, dx),
                                                       fvo, ALU.mult, ALU.add)

            def emit_head(s):
                _, _, featr = state.pop(s)
                h = hp.tile([128, 2, 640], f32r, tag="h")
                for og in range(2):
                    for off, w in ((0, 320), (320, 306)):
                        ph = psh.tile([128, 320], f32, tag="ph")
                        nc.tensor.matmul(ph[:, :w], w1t[:, 0 * 2 + og, :],
                                         featr[:, 0, off:off + w],
                                         start=True, stop=False)
                        nc.tensor.matmul(ph[:, :w], w1t[:, 1 * 2 + og, :],
                                         featr[:, 1, off:off + w],
                                         start=False, stop=True)
                        nc.scalar.activation(h[:, og, off:off + w], ph[:, :w], AF.Relu,
                                             scale=bnh[:, og:og + 1], bias=bnh[:, 2 + og:3 + og])

                out_s = hp.tile([128, 640], f32, tag="o")
                for off, w in ((0, 320), (320, 306)):
                    po = psh.tile([128, 320], f32, tag="ph")
                    nc.tensor.matmul(po[0:20, :w], w2t[:, 0, :], h[:, 0, off:off + w],
                                     start=True, stop=False)
                    nc.tensor.matmul(po[0:20, :w], w2t[:, 1, :], h[:, 1, off:off + w],
                                     start=False, stop=True)
                    nc.scalar.activation(out_s[0:20, off:off + w], po[0:20, :w],
                                         AF.Identity, bias=b2t[0:20, 0:1])
                nc.sync.dma_start(out=out_d[s, :, :], in_=out_s[0:20, 0:625])

            def emit_all():
                sin0 = sp.tile([128, 2, 968], bf16, tag="s_in")
                nc.scalar.dma_start(out=sin0, in_=search_d[0, :, :, :])
                state[("sin", 0)] = sin0
                emit_diags(0)
                emit_diags(1)
                for s in range(BPC):
                    emit_front(s)
                    if s + 2 < BPC:
                        emit_diags(s + 2)
                    if s >= 1:
                        emit_stt(s - 1)
                    if s >= 3:
                        emit_head(s - 3)
                emit_stt(BPC - 1)
                emit_head(BPC - 3)
                emit_head(BPC - 2)
                emit_head(BPC - 1)

            if bench_R:
                with tc.For_i(0, bench_R, 1,
                              hint_engines=(mybir.EngineType.PE,
                                            mybir.EngineType.DVE,
                                            mybir.EngineType.Activation)):
                    emit_all()
            else:
                emit_all()

    nc.compile()
    return nc


def _pack(inputs):
    import ml_dtypes
    f32 = np.float32
    bf16 = ml_dtypes.bfloat16
    kern = np.ascontiguousarray(inputs["kernel"], dtype=f32)
    search = np.ascontiguousarray(inputs["search"], dtype=f32)
    wk, ws = inputs["wk"].astype(f32), inputs["ws"].astype(f32)
    w1, w2, b2 = inputs["w1"].astype(f32), inputs["w2"].astype(f32), inputs["b2"].astype(f32)

    def fold(scale, bias, mean, var):
        inv = scale.astype(f32) / np.sqrt(var.astype(f32) + EPS)
        sh = bias.astype(f32) - mean.astype(f32) * inv
        arr = np.zeros((128, 4), f32)
        arr[:, 0:2] = inv.reshape(2, 128).T
        arr[:, 2:4] = sh.reshape(2, 128).T
        return arr

    bnk = fold(inputs["bnk_scale"], inputs["bnk_bias"], inputs["bnk_mean"], inputs["bnk_var"])
    bns = fold(inputs["bns_scale"], inputs["bns_bias"], inputs["bns_mean"], inputs["bns_var"])
    bnh = fold(inputs["bnh_scale"], inputs["bnh_bias"], inputs["bnh_mean"], inputs["bnh_var"])

    # conv weights -> lhsT tiles [ci, (cig,dy,dx,og), co], bf16
    def conv_w(w):
        w6 = w.reshape(2, 128, 2, 128, 3, 3)           # og co cig ci dy dx
        return np.ascontiguousarray(
            w6.transpose(3, 2, 4, 5, 0, 1).reshape(128, 36, 128).astype(bf16))

    wkt, wst = conv_w(wk), conv_w(ws)
    w1t = np.ascontiguousarray(
        w1[:, :, 0, 0].reshape(2, 128, 2, 128).transpose(3, 2, 0, 1).reshape(128, 4, 128))
    w2t = np.ascontiguousarray(
        w2[:, :, 0, 0].reshape(20, 2, 128).transpose(2, 1, 0))
    b2t = np.zeros((128, 1), f32)
    b2t[:20, 0] = b2
    ident = np.eye(128, dtype=f32)

    # search [64,256,31,31] -> per core [8, 128(ci), 2(cig), 961] bf16
    sr = np.zeros((NCORES, BPC, 128, 2, 968), bf16)
    sr[..., :961] = search.reshape(NCORES, BPC, 2, 128, 961).transpose(0, 1, 3, 2, 4).astype(bf16)
    # kernel [64,256,7,7] -> per core [128(ci), 2(cig), 8(s), 52] bf16
    kr = np.zeros((NCORES, 128, 2, BPC, 52), bf16)
    kr[..., :49] = kern.reshape(NCORES, BPC, 2, 128, 49).transpose(0, 3, 2, 1, 4).astype(bf16)

    in_maps = []
    for c in range(NCORES):
        in_maps.append({
            "search": np.ascontiguousarray(sr[c]),
            "tmpl": np.ascontiguousarray(kr[c]),
            "wkt": wkt, "wst": wst, "w1t": w1t, "w2t": w2t,
            "bnk": bnk, "bns": bns, "bnh": bnh, "b2t": b2t, "ident": ident,
        })
    return in_maps


def get_program(bench_R=0):
    key = f"nc{bench_R}"
    if key not in _CACHE:
        _CACHE[key] = _build(bench_R)
    return _CACHE[key]


def kernel(**inputs):
    from concourse.bass_utils import run_bass_kernel_spmd
    nc = get_program()
    in_maps = _pack(inputs)
    res = run_bass_kernel_spmd(nc, in_maps, core_ids=list(range(NCORES)))
    out = np.stack([res.results[c]["out"] for c in range(NCORES)], axis=0)
    return out.reshape(64, 20, 25, 25).astype(np.float32)
